# revision 5
# baseline (speedup 1.0000x reference)
"""Trainium2 Bass kernel for nn_CrossAttentionBlock_257698038372.

Strategy: data-parallel over batch (8 cores x 1 batch element). Per core:
  patch-embed (im2col DMA + PE transpose + matmul), maxpool via DVE,
  qkv with the token-pair reshape quirk resolved as column slices of qkv_w,
  per-head attention (logits -> fused exp+rowsum on ACT -> normalize ->
  PE-transpose -> attn@v), projection, DRAM-scratch flat reshape, 1x1 conv,
  bilinear x2 upsample as two matmuls against a host-built interpolation
  matrix, sigmoid, gate with context.
All matmuls in bf16 (f32 accumulate in PSUM).
"""

import numpy as np
import ml_dtypes

import concourse.bass as bass
import concourse.mybir as mybir
import concourse.tile as tile
from concourse import bacc
from concourse.bass_utils import run_bass_kernel_spmd
from concourse.masks import make_identity

bf16 = ml_dtypes.bfloat16
F32 = mybir.dt.float32
BF16 = mybir.dt.bfloat16
AF = mybir.ActivationFunctionType
OP = mybir.AluOpType

C, HIMG, WIMG, PATCH, E, HEADS = 3, 512, 512, 16, 768, 8
NPATCH = 1024          # patches per image (32x32)
NPAIR = 1024           # token pairs (quirk): 2048 tokens -> 1024 pairs
OUT_DIM = 384
SCALE = float((E // HEADS) ** -0.5)


def _pieces(h):
    """Partition-aligned (subtile, part_offset, size) pieces covering rows
    [192h, 192h+192) of a (128, nsub, ...) packed [row%128, row//128] tile."""
    a, r = divmod(192 * h, 128)
    if r == 0:
        return [(a, 0, 128), (a + 1, 0, 64)]
    return [(a, 64, 64), (a + 1, 0, 128)]


def _embed(nc, psE, psT, eb, ident, src_d, w_sb, pos_sb, b_sb, XeT, XoT, colbase):
    # im2col: patches on partitions [pa%128, pa//128, (c ky kx)]
    P = eb.tile([128, 8, E], BF16, tag="P", name="P")
    for t in range(8):
        for g in range(4):
            py = t * 4 + g
            for c in range(C):
                nc.sync.dma_start(
                    P[g * 32:(g + 1) * 32, t, c * 256:(c + 1) * 256].rearrange(
                        "p (ky kx) -> p ky kx", ky=16),
                    src_d[c, 16 * py:16 * py + 16, :].rearrange(
                        "ky (px kx) -> px ky kx", kx=16),
                )
    # transpose -> PT [k%128, k//128, pa]
    PT = eb.tile([128, 6, NPATCH], BF16, tag="PT", name="PT")
    for s in range(6):
        pt_ps = psT.tile([128, NPATCH], BF16, tag="ptps", name="ptps")
        for t in range(8):
            nc.tensor.transpose(
                pt_ps[:, t * 128:(t + 1) * 128],
                P[:, t, s * 128:(s + 1) * 128], ident[:])
        nc.any.tensor_copy(PT[:, s, :], pt_ps[:])
    # embed matmul: X^T[e, pa]; evict split into even/odd pair halves (+bias+pos)
    for m in range(6):
        ps = psE.tile([128, NPATCH], F32, tag="psemb", name="psemb")
        for ks in range(6):
            for n2 in range(2):
                nc.tensor.matmul(
                    ps[:, n2 * 512:(n2 + 1) * 512],
                    lhsT=w_sb[:, ks, m * 128:(m + 1) * 128],
                    rhs=PT[:, ks, n2 * 512:(n2 + 1) * 512],
                    start=(ks == 0), stop=(ks == 5))
        nc.vector.scalar_tensor_tensor(
            XeT[:, m, colbase:colbase + 512], ps[:, 0::2],
            b_sb[:, m:m + 1], pos_sb[:, m, 0::2], OP.add, OP.add)
        nc.vector.scalar_tensor_tensor(
            XoT[:, m, colbase:colbase + 512], ps[:, 1::2],
            b_sb[:, m:m + 1], pos_sb[:, m, 1::2], OP.add, OP.add)


def _build_nc():
    nc = bacc.Bacc(None, target_bir_lowering=False)

    # ---- I/O ----
    img_d = nc.dram_tensor("img", [C, HIMG, WIMG], BF16, kind="ExternalInput")
    ctxh_d = nc.dram_tensor("ctxh", [C, 2 * HIMG, 2 * WIMG], BF16, kind="ExternalInput")
    ctxf_d = nc.dram_tensor("ctxf", [C, 2 * HIMG, 2 * WIMG], F32, kind="ExternalInput")
    wp_d = nc.dram_tensor("wp", [128, 6, E], BF16, kind="ExternalInput")
    wc_d = nc.dram_tensor("wc", [128, 6, E], BF16, kind="ExternalInput")
    qkvw_d = nc.dram_tensor("qkvw", [128, 6, 3 * E], BF16, kind="ExternalInput")
    projw_d = nc.dram_tensor("projw", [128, 6, OUT_DIM], BF16, kind="ExternalInput")
    pos1t_d = nc.dram_tensor("pos1t", [128, 6, NPATCH], BF16, kind="ExternalInput")
    pos2t_d = nc.dram_tensor("pos2t", [128, 6, NPATCH], BF16, kind="ExternalInput")
    pb_d = nc.dram_tensor("pb", [128, 6], F32, kind="ExternalInput")
    cb_d = nc.dram_tensor("cb", [128, 6], F32, kind="ExternalInput")
    projb_d = nc.dram_tensor("projb", [128, OUT_DIM], F32, kind="ExternalInput")
    aht_d = nc.dram_tensor("aht", [128, 4, 1024], BF16, kind="ExternalInput")
    upw_d = nc.dram_tensor("upw", [128, 9], F32, kind="ExternalInput")
    upb_d = nc.dram_tensor("upb", [128, 3], F32, kind="ExternalInput")

    attn_d = nc.dram_tensor("attn", [HEADS, NPAIR, NPAIR], F32, kind="ExternalOutput")
    y_d = nc.dram_tensor("y", [C, 2 * HIMG, 2 * WIMG], F32, kind="ExternalOutput")

    with tile.TileContext(nc) as tc:
        with (
            tc.tile_pool(name="persist", bufs=1) as pp,
            tc.tile_pool(name="dram", bufs=1, space="DRAM") as dramp,
        ):
            ident = pp.tile([128, 128], BF16)
            make_identity(nc, ident)
            projw = pp.tile([128, 6, OUT_DIM], BF16)
            nc.sync.dma_start(projw[:], projw_d[:])
            aht = pp.tile([128, 4, 1024], BF16)
            nc.sync.dma_start(aht[:], aht_d[:])
            upw = pp.tile([128, 9], F32)
            nc.sync.dma_start(upw[:], upw_d[:])
            upb = pp.tile([128, 3], F32)
            nc.sync.dma_start(upb[:], upb_d[:])
            projb = pp.tile([128, OUT_DIM], F32)
            nc.sync.dma_start(projb[:], projb_d[:])
            # attention outputs (packed [e%128, e//128, pair]); even/odd tokens
            oTe = pp.tile([128, 6, NPAIR], BF16)
            oTo = pp.tile([128, 6, NPAIR], BF16)

            pooled_scr = dramp.tile([C, HIMG, WIMG], BF16)
            scr = dramp.tile([2048 * OUT_DIM], F32)

            # ---------- phase A1: maxpool context -> DRAM scratch ----------
            with tc.tile_pool(name="poolA", bufs=3) as pa:
                for c in range(C):
                    for yt in range(4):
                        tl = pa.tile([128, 2, 1024], BF16, tag="mpin", name="mpin")
                        nc.sync.dma_start(
                            tl[:],
                            ctxh_d[c, yt * 256:(yt + 1) * 256, :].rearrange(
                                "(p two) x -> p two x", two=2),
                        )
                        vm = pa.tile([128, 1024], BF16, tag="mpv", name="mpv")
                        nc.vector.tensor_tensor(vm[:], tl[:, 0, :], tl[:, 1, :], OP.max)
                        hm = pa.tile([128, 512], BF16, tag="mph", name="mph")
                        nc.vector.tensor_tensor(hm[:], vm[:, 0::2], vm[:, 1::2], OP.max)
                        nc.sync.dma_start(pooled_scr[c, yt * 128:(yt + 1) * 128, :], hm[:])

            # ---------- phases A2+A3+B under the attention-persistent pool ----------
            with tc.tile_pool(name="attnP", bufs=1) as ap_:
                QT = ap_.tile([128, 12, NPAIR], BF16)
                KT = ap_.tile([128, 12, NPAIR], BF16)
                V = ap_.tile([128, 8, 1536], BF16)

                with tc.tile_pool(name="mid", bufs=1) as midp:
                    XeT = midp.tile([128, 6, NPAIR], BF16)
                    XoT = midp.tile([128, 6, NPAIR], BF16)

                    # -- A2: patch embeds --
                    with (
                        tc.tile_pool(name="embw", bufs=1) as ew,
                        tc.tile_pool(name="emb", bufs=1) as eb,
                        tc.tile_pool(name="psE", bufs=2, space="PSUM") as psE,
                        tc.tile_pool(name="psT", bufs=2, space="PSUM") as psT,
                    ):
                        for src_d, w_d2, pos_d2, b_d2, colbase in (
                            (img_d, wp_d, pos1t_d, pb_d, 0),
                            (pooled_scr, wc_d, pos2t_d, cb_d, 512),
                        ):
                            w_sb = ew.tile([128, 6, E], BF16, tag="wemb", name="wemb")
                            nc.sync.dma_start(w_sb[:], w_d2[:])
                            pos_sb = ew.tile([128, 6, NPATCH], BF16, tag="pose", name="pose")
                            nc.sync.dma_start(pos_sb[:], pos_d2[:])
                            b_sb = ew.tile([128, 6], F32, tag="bemb", name="bemb")
                            nc.sync.dma_start(b_sb[:], b_d2[:])
                            _embed(nc, psE, psT, eb, ident, src_d, w_sb, pos_sb,
                                   b_sb, XeT, XoT, colbase)

                    # -- A3: qkv --
                    with (
                        tc.tile_pool(name="qkvp", bufs=1) as qw,
                        tc.tile_pool(name="psQ", bufs=2, space="PSUM") as psQ,
                        tc.tile_pool(name="psV", bufs=2, space="PSUM") as psV,
                    ):
                        qkvw = qw.tile([128, 6, 3 * E], BF16)
                        nc.sync.dma_start(qkvw[:], qkvw_d[:])

                        for dst, ft0, ftn, colf, rhs in (
                            (QT, 0, 12, lambda ft: ft * 128, XeT),
                            (KT, 0, 6, lambda ft: 1536 + ft * 128, XeT),
                            (KT, 6, 12, lambda ft: (ft - 6) * 128, XoT),
                        ):
                            for ft in range(ft0, ftn):
                                ps = psQ.tile([128, NPAIR], F32, tag="psqkv", name="psqkv")
                                for es in range(6):
                                    for n2 in range(2):
                                        nc.tensor.matmul(
                                            ps[:, n2 * 512:(n2 + 1) * 512],
                                            lhsT=qkvw[:, es, colf(ft):colf(ft) + 128],
                                            rhs=rhs[:, es, n2 * 512:(n2 + 1) * 512],
                                            start=(es == 0), stop=(es == 5))
                                nc.any.tensor_copy(dst[:, ft, :], ps[:])
                        # V[pair, f] = Xo @ qkv_w[:, 768:2304]
                        for pt in range(8):
                            for n3 in range(3):
                                ps = psV.tile([128, 512], F32, tag="psv", name="psv")
                                for es in range(6):
                                    nc.tensor.matmul(
                                        ps[:],
                                        lhsT=XoT[:, es, pt * 128:(pt + 1) * 128],
                                        rhs=qkvw[:, es, 768 + n3 * 512:768 + (n3 + 1) * 512],
                                        start=(es == 0), stop=(es == 5))
                                nc.any.tensor_copy(V[:, pt, n3 * 512:(n3 + 1) * 512], ps[:])

                # ---------- phase B: attention per head ----------
                with (
                    tc.tile_pool(name="attw", bufs=2) as aw,
                    tc.tile_pool(name="psL", bufs=2, space="PSUM") as psL,
                    tc.tile_pool(name="psT2", bufs=2, space="PSUM") as psT2,
                    tc.tile_pool(name="psO", bufs=2, space="PSUM") as psO,
                ):
                    for h in range(HEADS):
                        pieces = _pieces(h)
                        Et = aw.tile([128, 8, NPAIR], BF16, tag="E", name="E")
                        sums = aw.tile([128, 8], F32, tag="sums", name="sums")
                        rsum = aw.tile([128, 8], F32, tag="rsum", name="rsum")
                        for qt in range(8):
                            ps = psL.tile([128, NPAIR], F32, tag="psl", name="psl")
                            for pi, (s, po, sz) in enumerate(pieces):
                                for n2 in range(2):
                                    nc.tensor.matmul(
                                        ps[:, n2 * 512:(n2 + 1) * 512],
                                        lhsT=QT[po:po + sz, s, qt * 128:(qt + 1) * 128],
                                        rhs=KT[po:po + sz, s, n2 * 512:(n2 + 1) * 512],
                                        start=(pi == 0), stop=(pi == len(pieces) - 1))
                            nc.scalar.activation(
                                Et[:, qt, :], ps[:], AF.Exp, scale=SCALE,
                                accum_out=sums[:, qt:qt + 1])
                        nc.vector.reciprocal(rsum[:], sums[:])
                        for qt in range(8):
                            nc.vector.tensor_scalar_mul(
                                Et[:, qt, :], Et[:, qt, :], rsum[:, qt:qt + 1])
                            nc.gpsimd.dma_start(
                                attn_d[h, qt * 128:(qt + 1) * 128, :], Et[:, qt, :])
                        AT = aw.tile([128, 8, NPAIR], BF16, tag="AT", name="AT")
                        for kt in range(8):
                            pst = psT2.tile([128, NPAIR], BF16, tag="pst", name="pst")
                            for qt in range(8):
                                nc.tensor.transpose(
                                    pst[:, qt * 128:(qt + 1) * 128],
                                    Et[:, qt, kt * 128:(kt + 1) * 128], ident[:])
                            nc.any.tensor_copy(AT[:, kt, :], pst[:])
                        dst = oTe if h < 4 else oTo
                        e0h = 192 * (h % 4)
                        for moff, msz in ((0, 128), (128, 64)):
                            for n2 in range(2):
                                pso = psO.tile([128, 512], F32, tag="pso", name="pso")
                                for kt in range(8):
                                    nc.tensor.matmul(
                                        pso[:msz, :],
                                        lhsT=V[:, kt, 192 * h + moff:192 * h + moff + msz],
                                        rhs=AT[:, kt, n2 * 512:(n2 + 1) * 512],
                                        start=(kt == 0), stop=(kt == 7))
                                off = 0
                                while off < msz:
                                    e = e0h + moff + off
                                    s_, p_ = divmod(e, 128)
                                    chunk = min(msz - off, 128 - p_)
                                    nc.any.tensor_copy(
                                        dst[p_:p_ + chunk, s_, n2 * 512:(n2 + 1) * 512],
                                        pso[off:off + chunk, :])
                                    off += chunk

            # ---------- phase C: projection -> scratch ----------
            scrv = scr[:].rearrange("(n two d) -> n two d", two=2, d=OUT_DIM)
            with (
                tc.tile_pool(name="projp", bufs=3) as prp,
                tc.tile_pool(name="psP", bufs=2, space="PSUM") as psP,
            ):
                for half, par in ((oTe, 0), (oTo, 1)):
                    for qt in range(8):
                        ps = psP.tile([128, OUT_DIM], F32, tag="psp", name="psp")
                        for es in range(6):
                            nc.tensor.matmul(
                                ps[:], lhsT=half[:, es, qt * 128:(qt + 1) * 128],
                                rhs=projw[:, es, :], start=(es == 0), stop=(es == 5))
                        ob = prp.tile([128, OUT_DIM], F32, tag="ob", name="ob")
                        nc.vector.tensor_tensor(
                            ob[:], ps[:], projb[:], OP.add)
                        nc.sync.dma_start(scrv[qt * 128:(qt + 1) * 128, par, :], ob[:])

            # ---------- phase D: 1x1 conv + bilinear x2 + sigmoid + gate ----------
            scrp = scr[:].rearrange("(c h w) -> c h w", c=C, h=HIMG)
            with (
                tc.tile_pool(name="upsp", bufs=3) as up,
                tc.tile_pool(name="upc", bufs=1) as upc,
                tc.tile_pool(name="psH", bufs=2, space="PSUM") as psH,
                tc.tile_pool(name="psF", bufs=2, space="PSUM") as psF,
            ):
                ych = [upc.tile([128, 4, 512], BF16, tag=f"ych{c}", name=f"ych{c}")
                       for c in range(C)]
                for c in range(C):
                    nc.gpsimd.dma_start(
                        ych[c][:], scrp[c].rearrange("(s p) x -> p s x", p=128))
                yconv = [upc.tile([128, 4, 512], BF16, tag=f"yconv{c}", name=f"yconv{c}")
                         for c in range(C)]
                for co in range(C):
                    t0 = up.tile([128, 4, 512], F32, tag="convtmp", name="convtmp")
                    nc.vector.tensor_scalar(
                        t0[:], ych[0][:], upw[:, 3 * co:3 * co + 1],
                        upb[:, co:co + 1], OP.mult, OP.add)
                    nc.vector.scalar_tensor_tensor(
                        t0[:], ych[1][:], upw[:, 3 * co + 1:3 * co + 2], t0[:],
                        OP.mult, OP.add)
                    nc.vector.scalar_tensor_tensor(
                        yconv[co][:], ych[2][:], upw[:, 3 * co + 2:3 * co + 3], t0[:],
                        OP.mult, OP.add)
                outhT = [upc.tile([128, 4, 1024], BF16, tag=f"outhT{c}", name=f"outhT{c}")
                         for c in range(C)]
                for c in range(C):
                    for xt in range(4):
                        ps = psH.tile([128, 1024], F32, tag="psh", name="psh")
                        for ys in range(4):
                            for n2 in range(2):
                                nc.tensor.matmul(
                                    ps[:, n2 * 512:(n2 + 1) * 512],
                                    lhsT=yconv[c][:, ys, xt * 128:(xt + 1) * 128],
                                    rhs=aht[:, ys, n2 * 512:(n2 + 1) * 512],
                                    start=(ys == 0), stop=(ys == 3))
                        nc.any.tensor_copy(outhT[c][:, xt, :], ps[:])
                for c in range(C):
                    for jt in range(8):
                        ps = psF.tile([128, 1024], F32, tag="psf", name="psf")
                        for xs in range(4):
                            for n2 in range(2):
                                nc.tensor.matmul(
                                    ps[:, n2 * 512:(n2 + 1) * 512],
                                    lhsT=outhT[c][:, xs, jt * 128:(jt + 1) * 128],
                                    rhs=aht[:, xs, n2 * 512:(n2 + 1) * 512],
                                    start=(xs == 0), stop=(xs == 3))
                        sg = up.tile([128, 1024], BF16, tag="sg", name="sg")
                        nc.scalar.activation(sg[:], ps[:], AF.Sigmoid)
                        cx = up.tile([128, 1024], F32, tag="cx", name="cx")
                        nc.sync.dma_start(cx[:], ctxf_d[c, jt * 128:(jt + 1) * 128, :])
                        yo = up.tile([128, 1024], F32, tag="yo", name="yo")
                        nc.vector.tensor_tensor(yo[:], sg[:], cx[:], OP.mult)
                        nc.sync.dma_start(y_d[c, jt * 128:(jt + 1) * 128, :], yo[:])

    nc.compile()
    return nc


_NC_CACHE = None
_LAST_IN_MAPS = None


def _kxm_pack(a, dtype=bf16):
    """(K, M) -> (128, K//128, M) partition-major."""
    K, M = a.shape
    return np.ascontiguousarray(
        a.reshape(K // 128, 128, M).transpose(1, 0, 2)).astype(dtype)


def _upsample_matrix():
    H, s = 512, 2
    ys = np.linspace(0.0, H - 1.0, H * s)
    y0 = np.floor(ys).astype(np.int64)
    y1 = np.minimum(y0 + 1, H - 1)
    w = ys - y0
    A = np.zeros((H * s, H), np.float64)
    A[np.arange(H * s), y0] += 1.0 - w
    A[np.arange(H * s), y1] += w
    return A.T.astype(np.float32)  # (512, 1024) = A^T


def kernel(img, context, patch_w, patch_b, pos1, ctx_w, ctx_b, pos2,
           qkv_w, proj_w, proj_b, up_w, up_b):
    global _NC_CACHE, _LAST_IN_MAPS
    if _NC_CACHE is None:
        _NC_CACHE = _build_nc()
    nc = _NC_CACHE

    B = img.shape[0]
    img = np.asarray(img, np.float32)
    context = np.asarray(context, np.float32)

    shared = {
        "wp": _kxm_pack(np.asarray(patch_w, np.float32).reshape(E, E).T),
        "wc": _kxm_pack(np.asarray(ctx_w, np.float32).reshape(E, E).T),
        "qkvw": _kxm_pack(np.asarray(qkv_w, np.float32)),
        "projw": _kxm_pack(np.asarray(proj_w, np.float32)),
        "pos1t": _kxm_pack(np.ascontiguousarray(np.asarray(pos1, np.float32)[0].T)),
        "pos2t": _kxm_pack(np.ascontiguousarray(np.asarray(pos2, np.float32)[0].T)),
        "pb": np.ascontiguousarray(
            np.asarray(patch_b, np.float32).reshape(6, 128).T),
        "cb": np.ascontiguousarray(
            np.asarray(ctx_b, np.float32).reshape(6, 128).T),
        "projb": np.tile(np.asarray(proj_b, np.float32).reshape(1, OUT_DIM), (128, 1)),
        "aht": _kxm_pack(_upsample_matrix()),
        "upw": np.tile(np.asarray(up_w, np.float32).reshape(1, 9), (128, 1)),
        "upb": np.tile(np.asarray(up_b, np.float32).reshape(1, 3), (128, 1)),
    }
    in_maps = []
    for b in range(B):
        m = dict(shared)
        m["img"] = np.ascontiguousarray(img[b]).astype(bf16)
        m["ctxh"] = np.ascontiguousarray(context[b]).astype(bf16)
        m["ctxf"] = np.ascontiguousarray(context[b])
        in_maps.append(m)
    _LAST_IN_MAPS = in_maps

    res = run_bass_kernel_spmd(nc, in_maps, core_ids=list(range(B)))
    y = np.stack([res.results[b]["y"] for b in range(B)], axis=0)
    attn = np.stack([res.results[b]["attn"] for b in range(B)], axis=0)
    return (y, attn)


# revision 6
# speedup vs baseline: 1.1266x; 1.1266x over previous
"""Trainium2 Bass kernel for nn_CrossAttentionBlock_257698038372.

Strategy: data-parallel over batch (8 cores x 1 batch element). Per core:
  patch-embed (im2col DMA + PE transpose + matmul), maxpool via DVE,
  qkv with the token-pair reshape quirk resolved as column slices of qkv_w,
  per-head attention (logits -> fused exp+rowsum on ACT -> normalize ->
  PE-transpose -> attn@v), projection, DRAM-scratch flat reshape, 1x1 conv,
  bilinear x2 upsample as two matmuls against a host-built interpolation
  matrix, sigmoid, gate with context.
All matmuls in bf16 (f32 accumulate in PSUM).
"""

import numpy as np
import ml_dtypes

import concourse.bass as bass
import concourse.mybir as mybir
import concourse.tile as tile
from concourse import bacc
from concourse.bass_utils import run_bass_kernel_spmd
from concourse.masks import make_identity

bf16 = ml_dtypes.bfloat16
F32 = mybir.dt.float32
BF16 = mybir.dt.bfloat16
AF = mybir.ActivationFunctionType
OP = mybir.AluOpType

C, HIMG, WIMG, PATCH, E, HEADS = 3, 512, 512, 16, 768, 8
NPATCH = 1024          # patches per image (32x32)
NPAIR = 1024           # token pairs (quirk): 2048 tokens -> 1024 pairs
OUT_DIM = 384
SCALE = float((E // HEADS) ** -0.5)


def _pieces(h):
    """Partition-aligned (subtile, part_offset, size) pieces covering rows
    [192h, 192h+192) of a (128, nsub, ...) packed [row%128, row//128] tile."""
    a, r = divmod(192 * h, 128)
    if r == 0:
        return [(a, 0, 128), (a + 1, 0, 64)]
    return [(a, 64, 64), (a + 1, 0, 128)]


def _embed(nc, psE, psT, eb, ident, src_d, w_sb, pos_sb, b_sb, XeT, XoT, colbase,
           direct=False):
    # im2col: patches on partitions [pa%128, pa//128, (c ky kx)]
    P = eb.tile([128, 8, E], BF16, tag="P", name="P")
    if direct:
        nc.sync.dma_start(P[:], src_d[:])
    else:
        for t in range(8):
            for g in range(4):
                py = t * 4 + g
                for c in range(C):
                    eng = nc.sync if (g + c) % 2 == 0 else nc.scalar
                    eng.dma_start(
                        P[g * 32:(g + 1) * 32, t, c * 256:(c + 1) * 256].rearrange(
                            "p (ky kx) -> p ky kx", ky=16),
                        src_d[c, 16 * py:16 * py + 16, :].rearrange(
                            "ky (px kx) -> px ky kx", kx=16),
                    )
    # transpose -> PT [k%128, k//128, pa]
    PT = eb.tile([128, 6, NPATCH], BF16, tag="PT", name="PT")
    for s in range(6):
        pt_ps = psT.tile([128, NPATCH], BF16, tag="ptps", name="ptps")
        for t in range(8):
            nc.tensor.transpose(
                pt_ps[:, t * 128:(t + 1) * 128],
                P[:, t, s * 128:(s + 1) * 128], ident[:])
        nc.any.tensor_copy(PT[:, s, :], pt_ps[:])
    # embed matmul: X^T[e, pa]; evict split into even/odd pair halves (+bias+pos)
    for m in range(6):
        ps = psE.tile([128, NPATCH], F32, tag="psemb", name="psemb")
        for ks in range(6):
            for n2 in range(2):
                nc.tensor.matmul(
                    ps[:, n2 * 512:(n2 + 1) * 512],
                    lhsT=w_sb[:, ks, m * 128:(m + 1) * 128],
                    rhs=PT[:, ks, n2 * 512:(n2 + 1) * 512],
                    start=(ks == 0), stop=(ks == 5))
        nc.vector.scalar_tensor_tensor(
            XeT[:, m, colbase:colbase + 512], ps[:, 0::2],
            b_sb[:, m:m + 1], pos_sb[:, m, 0::2], OP.add, OP.add)
        nc.vector.scalar_tensor_tensor(
            XoT[:, m, colbase:colbase + 512], ps[:, 1::2],
            b_sb[:, m:m + 1], pos_sb[:, m, 1::2], OP.add, OP.add)


def _build_nc():
    nc = bacc.Bacc(None, target_bir_lowering=False)

    # ---- I/O ----
    pimg_d = nc.dram_tensor("pimg", [128, 8, E], BF16, kind="ExternalInput")
    ctxh_d = nc.dram_tensor("ctxh", [C, 2 * HIMG, 2 * WIMG], BF16, kind="ExternalInput")
    ctxf_d = nc.dram_tensor("ctxf", [C, 2 * HIMG, 2 * WIMG], F32, kind="ExternalInput")
    wp_d = nc.dram_tensor("wp", [128, 6, E], BF16, kind="ExternalInput")
    wc_d = nc.dram_tensor("wc", [128, 6, E], BF16, kind="ExternalInput")
    qkvw_d = nc.dram_tensor("qkvw", [128, 6, 3 * E], BF16, kind="ExternalInput")
    projw_d = nc.dram_tensor("projw", [128, 6, OUT_DIM], BF16, kind="ExternalInput")
    pos1t_d = nc.dram_tensor("pos1t", [128, 6, NPATCH], BF16, kind="ExternalInput")
    pos2t_d = nc.dram_tensor("pos2t", [128, 6, NPATCH], BF16, kind="ExternalInput")
    pb_d = nc.dram_tensor("pb", [128, 6], F32, kind="ExternalInput")
    cb_d = nc.dram_tensor("cb", [128, 6], F32, kind="ExternalInput")
    projb_d = nc.dram_tensor("projb", [128, OUT_DIM], F32, kind="ExternalInput")
    aht_d = nc.dram_tensor("aht", [128, 4, 1024], BF16, kind="ExternalInput")
    upw_d = nc.dram_tensor("upw", [128, 9], F32, kind="ExternalInput")
    upb_d = nc.dram_tensor("upb", [128, 3], F32, kind="ExternalInput")

    attn_d = nc.dram_tensor("attn", [HEADS, NPAIR, NPAIR], F32, kind="ExternalOutput")
    y_d = nc.dram_tensor("y", [C, 2 * HIMG, 2 * WIMG], F32, kind="ExternalOutput")

    with tile.TileContext(nc) as tc:
        with (
            tc.tile_pool(name="persist", bufs=1) as pp,
            tc.tile_pool(name="dram", bufs=1, space="DRAM") as dramp,
        ):
            ident = pp.tile([128, 128], BF16)
            make_identity(nc, ident)
            projw = pp.tile([128, 6, OUT_DIM], BF16)
            nc.sync.dma_start(projw[:], projw_d[:])
            aht = pp.tile([128, 4, 1024], BF16)
            nc.sync.dma_start(aht[:], aht_d[:])
            upw = pp.tile([128, 9], F32)
            nc.sync.dma_start(upw[:], upw_d[:])
            upb = pp.tile([128, 3], F32)
            nc.sync.dma_start(upb[:], upb_d[:])
            projb = pp.tile([128, OUT_DIM], F32)
            nc.sync.dma_start(projb[:], projb_d[:])
            # attention outputs (packed [e%128, e//128, pair]); even/odd tokens
            oTe = pp.tile([128, 6, NPAIR], BF16)
            oTo = pp.tile([128, 6, NPAIR], BF16)

            pooled_scr = dramp.tile([C, HIMG, WIMG], BF16)
            scr = dramp.tile([2048 * OUT_DIM], F32)

            # ---------- phase A1: maxpool context -> DRAM scratch ----------
            with tc.tile_pool(name="poolA", bufs=3) as pa:
                for c in range(C):
                    for yt in range(4):
                        tl = pa.tile([128, 2, 1024], BF16, tag="mpin", name="mpin")
                        eng = nc.sync if (c + yt) % 2 == 0 else nc.scalar
                        eng.dma_start(
                            tl[:],
                            ctxh_d[c, yt * 256:(yt + 1) * 256, :].rearrange(
                                "(p two) x -> p two x", two=2),
                        )
                        vm = pa.tile([128, 1024], BF16, tag="mpv", name="mpv")
                        nc.vector.tensor_tensor(vm[:], tl[:, 0, :], tl[:, 1, :], OP.max)
                        hm = pa.tile([128, 512], BF16, tag="mph", name="mph")
                        nc.vector.tensor_tensor(hm[:], vm[:, 0::2], vm[:, 1::2], OP.max)
                        nc.sync.dma_start(pooled_scr[c, yt * 128:(yt + 1) * 128, :], hm[:])

            # ---------- phases A2+A3+B under the attention-persistent pool ----------
            with tc.tile_pool(name="attnP", bufs=1) as ap_:
                QT = ap_.tile([128, 12, NPAIR], BF16)
                KT = ap_.tile([128, 12, NPAIR], BF16)
                V = ap_.tile([128, 8, 1536], BF16)

                with tc.tile_pool(name="mid", bufs=1) as midp:
                    XeT = midp.tile([128, 6, NPAIR], BF16)
                    XoT = midp.tile([128, 6, NPAIR], BF16)

                    # -- A2: patch embeds --
                    with (
                        tc.tile_pool(name="embw", bufs=1) as ew,
                        tc.tile_pool(name="emb", bufs=1) as eb,
                        tc.tile_pool(name="psE", bufs=2, space="PSUM") as psE,
                        tc.tile_pool(name="psT", bufs=2, space="PSUM") as psT,
                    ):
                        for src_d, w_d2, pos_d2, b_d2, colbase, direct in (
                            (pimg_d, wp_d, pos1t_d, pb_d, 0, True),
                            (pooled_scr, wc_d, pos2t_d, cb_d, 512, False),
                        ):
                            w_sb = ew.tile([128, 6, E], BF16, tag="wemb", name="wemb")
                            nc.sync.dma_start(w_sb[:], w_d2[:])
                            pos_sb = ew.tile([128, 6, NPATCH], BF16, tag="pose", name="pose")
                            nc.sync.dma_start(pos_sb[:], pos_d2[:])
                            b_sb = ew.tile([128, 6], F32, tag="bemb", name="bemb")
                            nc.sync.dma_start(b_sb[:], b_d2[:])
                            _embed(nc, psE, psT, eb, ident, src_d, w_sb, pos_sb,
                                   b_sb, XeT, XoT, colbase, direct=direct)

                    # -- A3: qkv --
                    with (
                        tc.tile_pool(name="qkvp", bufs=1) as qw,
                        tc.tile_pool(name="psQ", bufs=2, space="PSUM") as psQ,
                        tc.tile_pool(name="psV", bufs=2, space="PSUM") as psV,
                    ):
                        qkvw = qw.tile([128, 6, 3 * E], BF16)
                        nc.sync.dma_start(qkvw[:], qkvw_d[:])

                        for dst, ft0, ftn, colf, rhs in (
                            (QT, 0, 12, lambda ft: ft * 128, XeT),
                            (KT, 0, 6, lambda ft: 1536 + ft * 128, XeT),
                            (KT, 6, 12, lambda ft: (ft - 6) * 128, XoT),
                        ):
                            for ft in range(ft0, ftn):
                                ps = psQ.tile([128, NPAIR], F32, tag="psqkv", name="psqkv")
                                for es in range(6):
                                    for n2 in range(2):
                                        nc.tensor.matmul(
                                            ps[:, n2 * 512:(n2 + 1) * 512],
                                            lhsT=qkvw[:, es, colf(ft):colf(ft) + 128],
                                            rhs=rhs[:, es, n2 * 512:(n2 + 1) * 512],
                                            start=(es == 0), stop=(es == 5))
                                nc.any.tensor_copy(dst[:, ft, :], ps[:])
                        # V[pair, f] = Xo @ qkv_w[:, 768:2304]
                        for pt in range(8):
                            for n3 in range(3):
                                ps = psV.tile([128, 512], F32, tag="psv", name="psv")
                                for es in range(6):
                                    nc.tensor.matmul(
                                        ps[:],
                                        lhsT=XoT[:, es, pt * 128:(pt + 1) * 128],
                                        rhs=qkvw[:, es, 768 + n3 * 512:768 + (n3 + 1) * 512],
                                        start=(es == 0), stop=(es == 5))
                                nc.any.tensor_copy(V[:, pt, n3 * 512:(n3 + 1) * 512], ps[:])

                # ---------- phase B: attention per head ----------
                with (
                    tc.tile_pool(name="attw", bufs=2) as aw,
                    tc.tile_pool(name="psL", bufs=2, space="PSUM") as psL,
                    tc.tile_pool(name="psT2", bufs=2, space="PSUM") as psT2,
                    tc.tile_pool(name="psO", bufs=2, space="PSUM") as psO,
                ):
                    for h in range(HEADS):
                        pieces = _pieces(h)
                        Et = aw.tile([128, 8, NPAIR], BF16, tag="E", name="E")
                        sums = aw.tile([128, 8], F32, tag="sums", name="sums")
                        rsum = aw.tile([128, 8], F32, tag="rsum", name="rsum")
                        for qt in range(8):
                            ps = psL.tile([128, NPAIR], F32, tag="psl", name="psl")
                            for pi, (s, po, sz) in enumerate(pieces):
                                for n2 in range(2):
                                    nc.tensor.matmul(
                                        ps[:, n2 * 512:(n2 + 1) * 512],
                                        lhsT=QT[po:po + sz, s, qt * 128:(qt + 1) * 128],
                                        rhs=KT[po:po + sz, s, n2 * 512:(n2 + 1) * 512],
                                        start=(pi == 0), stop=(pi == len(pieces) - 1))
                            nc.scalar.activation(
                                Et[:, qt, :], ps[:], AF.Exp, scale=SCALE,
                                accum_out=sums[:, qt:qt + 1])
                        nc.vector.reciprocal(rsum[:], sums[:])
                        for qt in range(8):
                            nc.vector.tensor_scalar_mul(
                                Et[:, qt, :], Et[:, qt, :], rsum[:, qt:qt + 1])
                            nc.gpsimd.dma_start(
                                attn_d[h, qt * 128:(qt + 1) * 128, :], Et[:, qt, :])
                        AT = aw.tile([128, 8, NPAIR], BF16, tag="AT", name="AT")
                        for kt in range(8):
                            pst = psT2.tile([128, NPAIR], BF16, tag="pst", name="pst")
                            for qt in range(8):
                                nc.tensor.transpose(
                                    pst[:, qt * 128:(qt + 1) * 128],
                                    Et[:, qt, kt * 128:(kt + 1) * 128], ident[:])
                            nc.any.tensor_copy(AT[:, kt, :], pst[:])
                        dst = oTe if h < 4 else oTo
                        e0h = 192 * (h % 4)
                        for moff, msz in ((0, 128), (128, 64)):
                            for n2 in range(2):
                                pso = psO.tile([128, 512], F32, tag="pso", name="pso")
                                for kt in range(8):
                                    nc.tensor.matmul(
                                        pso[:msz, :],
                                        lhsT=V[:, kt, 192 * h + moff:192 * h + moff + msz],
                                        rhs=AT[:, kt, n2 * 512:(n2 + 1) * 512],
                                        start=(kt == 0), stop=(kt == 7))
                                off = 0
                                while off < msz:
                                    e = e0h + moff + off
                                    s_, p_ = divmod(e, 128)
                                    chunk = min(msz - off, 128 - p_)
                                    nc.any.tensor_copy(
                                        dst[p_:p_ + chunk, s_, n2 * 512:(n2 + 1) * 512],
                                        pso[off:off + chunk, :])
                                    off += chunk

            # ---------- phase C: projection -> scratch ----------
            scrv = scr[:].rearrange("(n two d) -> n two d", two=2, d=OUT_DIM)
            with (
                tc.tile_pool(name="projp", bufs=3) as prp,
                tc.tile_pool(name="psP", bufs=2, space="PSUM") as psP,
            ):
                for half, par in ((oTe, 0), (oTo, 1)):
                    for qt in range(8):
                        ps = psP.tile([128, OUT_DIM], F32, tag="psp", name="psp")
                        for es in range(6):
                            nc.tensor.matmul(
                                ps[:], lhsT=half[:, es, qt * 128:(qt + 1) * 128],
                                rhs=projw[:, es, :], start=(es == 0), stop=(es == 5))
                        ob = prp.tile([128, OUT_DIM], F32, tag="ob", name="ob")
                        nc.vector.tensor_tensor(
                            ob[:], ps[:], projb[:], OP.add)
                        nc.sync.dma_start(scrv[qt * 128:(qt + 1) * 128, par, :], ob[:])

            # ---------- phase D: 1x1 conv + bilinear x2 + sigmoid + gate ----------
            scrp = scr[:].rearrange("(c h w) -> c h w", c=C, h=HIMG)
            with (
                tc.tile_pool(name="upsp", bufs=3) as up,
                tc.tile_pool(name="upc", bufs=1) as upc,
                tc.tile_pool(name="psH", bufs=2, space="PSUM") as psH,
                tc.tile_pool(name="psF", bufs=2, space="PSUM") as psF,
            ):
                ych = [upc.tile([128, 4, 512], BF16, tag=f"ych{c}", name=f"ych{c}")
                       for c in range(C)]
                for c in range(C):
                    nc.gpsimd.dma_start(
                        ych[c][:], scrp[c].rearrange("(s p) x -> p s x", p=128))
                yconv = [upc.tile([128, 4, 512], BF16, tag=f"yconv{c}", name=f"yconv{c}")
                         for c in range(C)]
                for co in range(C):
                    t0 = up.tile([128, 4, 512], F32, tag="convtmp", name="convtmp")
                    nc.vector.tensor_scalar(
                        t0[:], ych[0][:], upw[:, 3 * co:3 * co + 1],
                        upb[:, co:co + 1], OP.mult, OP.add)
                    nc.vector.scalar_tensor_tensor(
                        t0[:], ych[1][:], upw[:, 3 * co + 1:3 * co + 2], t0[:],
                        OP.mult, OP.add)
                    nc.vector.scalar_tensor_tensor(
                        yconv[co][:], ych[2][:], upw[:, 3 * co + 2:3 * co + 3], t0[:],
                        OP.mult, OP.add)
                outhT = [upc.tile([128, 4, 1024], BF16, tag=f"outhT{c}", name=f"outhT{c}")
                         for c in range(C)]
                for c in range(C):
                    for xt in range(4):
                        ps = psH.tile([128, 1024], F32, tag="psh", name="psh")
                        for ys in range(4):
                            for n2 in range(2):
                                nc.tensor.matmul(
                                    ps[:, n2 * 512:(n2 + 1) * 512],
                                    lhsT=yconv[c][:, ys, xt * 128:(xt + 1) * 128],
                                    rhs=aht[:, ys, n2 * 512:(n2 + 1) * 512],
                                    start=(ys == 0), stop=(ys == 3))
                        nc.any.tensor_copy(outhT[c][:, xt, :], ps[:])
                for c in range(C):
                    for jt in range(8):
                        ps = psF.tile([128, 1024], F32, tag="psf", name="psf")
                        for xs in range(4):
                            for n2 in range(2):
                                nc.tensor.matmul(
                                    ps[:, n2 * 512:(n2 + 1) * 512],
                                    lhsT=outhT[c][:, xs, jt * 128:(jt + 1) * 128],
                                    rhs=aht[:, xs, n2 * 512:(n2 + 1) * 512],
                                    start=(xs == 0), stop=(xs == 3))
                        sg = up.tile([128, 1024], BF16, tag="sg", name="sg")
                        nc.scalar.activation(sg[:], ps[:], AF.Sigmoid)
                        cx = up.tile([128, 1024], F32, tag="cx", name="cx")
                        nc.scalar.dma_start(cx[:], ctxf_d[c, jt * 128:(jt + 1) * 128, :])
                        yo = up.tile([128, 1024], F32, tag="yo", name="yo")
                        nc.vector.tensor_tensor(yo[:], sg[:], cx[:], OP.mult)
                        nc.sync.dma_start(y_d[c, jt * 128:(jt + 1) * 128, :], yo[:])

    nc.compile()
    return nc


_NC_CACHE = None
_LAST_IN_MAPS = None


def _kxm_pack(a, dtype=bf16):
    """(K, M) -> (128, K//128, M) partition-major."""
    K, M = a.shape
    return np.ascontiguousarray(
        a.reshape(K // 128, 128, M).transpose(1, 0, 2)).astype(dtype)


def _upsample_matrix():
    H, s = 512, 2
    ys = np.linspace(0.0, H - 1.0, H * s)
    y0 = np.floor(ys).astype(np.int64)
    y1 = np.minimum(y0 + 1, H - 1)
    w = ys - y0
    A = np.zeros((H * s, H), np.float64)
    A[np.arange(H * s), y0] += 1.0 - w
    A[np.arange(H * s), y1] += w
    return A.T.astype(np.float32)  # (512, 1024) = A^T


def kernel(img, context, patch_w, patch_b, pos1, ctx_w, ctx_b, pos2,
           qkv_w, proj_w, proj_b, up_w, up_b):
    global _NC_CACHE, _LAST_IN_MAPS
    if _NC_CACHE is None:
        _NC_CACHE = _build_nc()
    nc = _NC_CACHE

    B = img.shape[0]
    img = np.asarray(img, np.float32)
    context = np.asarray(context, np.float32)

    shared = {
        "wp": _kxm_pack(np.asarray(patch_w, np.float32).reshape(E, E).T),
        "wc": _kxm_pack(np.asarray(ctx_w, np.float32).reshape(E, E).T),
        "qkvw": _kxm_pack(np.asarray(qkv_w, np.float32)),
        "projw": _kxm_pack(np.asarray(proj_w, np.float32)),
        "pos1t": _kxm_pack(np.ascontiguousarray(np.asarray(pos1, np.float32)[0].T)),
        "pos2t": _kxm_pack(np.ascontiguousarray(np.asarray(pos2, np.float32)[0].T)),
        "pb": np.ascontiguousarray(
            np.asarray(patch_b, np.float32).reshape(6, 128).T),
        "cb": np.ascontiguousarray(
            np.asarray(ctx_b, np.float32).reshape(6, 128).T),
        "projb": np.tile(np.asarray(proj_b, np.float32).reshape(1, OUT_DIM), (128, 1)),
        "aht": _kxm_pack(_upsample_matrix()),
        "upw": np.tile(np.asarray(up_w, np.float32).reshape(1, 9), (128, 1)),
        "upb": np.tile(np.asarray(up_b, np.float32).reshape(1, 3), (128, 1)),
    }
    # host im2col for img: (C,512,512) -> patches (1024, 768) -> [pa%128, pa//128, k]
    imgb = img.astype(bf16)
    pimg_all = (imgb.reshape(B, C, 32, 16, 32, 16)
                .transpose(0, 2, 4, 1, 3, 5)        # (B, py, px, c, ky, kx)
                .reshape(B, 1024, E)
                .reshape(B, 8, 128, E)
                .transpose(0, 2, 1, 3))             # (B, 128, 8, 768)
    in_maps = []
    for b in range(B):
        m = dict(shared)
        m["pimg"] = np.ascontiguousarray(pimg_all[b])
        m["ctxh"] = np.ascontiguousarray(context[b]).astype(bf16)
        m["ctxf"] = np.ascontiguousarray(context[b])
        in_maps.append(m)
    _LAST_IN_MAPS = in_maps

    res = run_bass_kernel_spmd(nc, in_maps, core_ids=list(range(B)))
    y = np.stack([res.results[b]["y"] for b in range(B)], axis=0)
    attn = np.stack([res.results[b]["attn"] for b in range(B)], axis=0)
    return (y, attn)
